# revision 28
# baseline (speedup 1.0000x reference)
"""Trainium2 Bass kernel for nn_MemoryAttention (causal single-head attention
with SiLU-gated output projection), sequence-parallel across 8 NeuronCores.

Strategy (per core c):
  - q rows owned: 4 slots of 256 rows: tile t = c + 8*s (strided assignment
    balances causal work; every core runs an identical instruction stream).
  - fp8e4 (DoubleRow, 2x PE rate) for the Q/K projections and the QK^T
    logits: softmax logits are tiny (~+-0.1 after 1/32 scaling), so ~5%
    quantization on q/k perturbs attention weights by well under 1%.
    wq/wk are pre-scaled by 64 host-side (w std 0.01 would land in the
    fp8 subnormal range); the exp() scale folds the 64*64 back out.
  - Interior ("phase A", fully-visible) visits also run PV in fp8
    DoubleRow via the delta decomposition P = 1 + delta: h = prefix
    column-sums of V (exact, injected as rank-1 seed matmuls from
    colsum(X) @ wv1 colsums that ride the gather) + delta @ V8.  fp8
    errors then scale by |delta| ~ 0.15 instead of 1.  Boundary
    ("phase B", last 16 visits of each slot, masked) visits stay bf16.
  - Each core projects KT(fp8)/V(bf16+fp8) for its own tile of slot-level
    g, then immediately AllGathers that slot-level (4 small pipelined
    collectives) so slot g's kv arrives while earlier slots compute.
    Slot-level 0 stays SBUF-resident (serves the first 16 visits of
    every slot).
  - Slot epilogue: H / rowsums, SiLU, PE-transpose of G, output proj.
"""

import numpy as np
import ml_dtypes

import concourse.bass as bass
import concourse.tile as tile
from concourse import bacc, mybir
from concourse.bass_utils import run_bass_kernel_spmd
from concourse.masks import make_identity

P = 128
D = 1024
SEQ = 8192
NCORES = 8
NSLOTS = 4
WSCALE = 64.0
EXP_SCALE = 0.03125 / (WSCALE * WSCALE)

# kv payload rows (units of [., 1024] bf16): kt fp8 | v bf16 | v fp8 | colsums
R_KT = 0
R_V16 = P
R_V8 = 3 * P
R_CS = 4 * P
R_TOT = 4 * P + 2

F32 = mybir.dt.float32
BF16 = mybir.dt.bfloat16
FP8 = mybir.dt.float8e4
AF = mybir.ActivationFunctionType
DR = mybir.MatmulPerfMode.DoubleRow
AX = mybir.AxisListType
ALU = mybir.AluOpType


def build_kernel():
    nc = bacc.Bacc(None, target_bir_lowering=False, num_devices=NCORES)

    xb_ext = nc.declare_dram_parameter("xb", [D, D], BF16, isOutput=False)
    x8_ext = nc.declare_dram_parameter("x8", [D, D], FP8, isOutput=False)
    wq_ext = nc.declare_dram_parameter("wq", [D, D], FP8, isOutput=False)
    wk_ext = nc.declare_dram_parameter("wk", [D, D], FP8, isOutput=False)
    wv1_ext = nc.declare_dram_parameter("wv1", [D, D], BF16, isOutput=False)
    wv2_ext = nc.declare_dram_parameter("wv2", [D, D], BF16, isOutput=False)
    mask_ext = nc.declare_dram_parameter("masks", [64, P, 256], BF16, isOutput=False)
    o_ext = nc.declare_dram_parameter("o", [NSLOTS, 2, P, D], F32, isOutput=True)

    kv_local = nc.dram_tensor("kv_local", [NSLOTS, R_TOT, D], BF16)
    kv_gath = nc.dram_tensor(
        "kv_gath", [NSLOTS, NCORES, R_TOT, D], BF16, addr_space="Shared"
    )

    def wload(nc, pool, ext, tag, dt):
        t = pool.tile([P, 8, D], dt, tag=tag, name=tag)
        nc.sync.dma_start(out=t, in_=ext[:].rearrange("(sub p) s -> p sub s", p=P))
        return t

    with tile.TileContext(nc) as tc:
        singles_ctx = tc.tile_pool(name="singles", bufs=1)
        singles = singles_ctx.__enter__()

        ones_sb = singles.tile([P, 1], BF16)
        nc.vector.memset(ones_sb, 1.0)
        ones_row = singles.tile([1, P], BF16)
        nc.vector.memset(ones_row, 1.0)
        ones8_2 = singles.tile([P, 2, 1], FP8)
        nc.vector.memset(ones8_2, 1.0)
        cnt_sb = singles.tile([1, 2 * NSLOTS], BF16)
        for s in range(NSLOTS):
            nc.vector.memset(cnt_sb[:, 2 * s : 2 * s + 2], float(2048 * s))
        ident_sb = singles.tile([P, P], BF16)
        make_identity(nc, ident_sb)
        qt_sb = singles.tile([P, 8, D], FP8)
        xsumb = singles.tile([P, 8, 8], BF16)

        with (
            tc.tile_pool(name="projw", bufs=1) as projw,
            tc.tile_pool(name="projout", bufs=4) as projout,
            tc.tile_pool(name="ppsum", bufs=1, space="PSUM") as ppsum,
            tc.tile_pool(name="vpsum", bufs=4, space="PSUM") as vpsum,
            tc.tile_pool(name="cpsum", bufs=1, space="PSUM") as cpsum,
        ):
            # sub-pair-chunked loads so the first DoubleRow matmuls (which
            # consume one sub-pair at a time) start after two small DMAs
            def pairload(pool, ext, tag, dt):
                t = pool.tile([P, 8, D], dt, tag=tag, name=tag)
                v = ext[:].rearrange("(sub p) s -> p sub s", p=P)
                for p4 in range(4):
                    nc.sync.dma_start(
                        out=t[:, 2 * p4 : 2 * p4 + 2, :], in_=v[:, 2 * p4 : 2 * p4 + 2, :]
                    )
                return t

            x8_v = x8_ext[:].rearrange("(sub p) s -> p sub s", p=P)
            wk_v = wk_ext[:].rearrange("(sub p) s -> p sub s", p=P)
            x8_sb = projw.tile([P, 8, D], FP8, tag="x8", name="x8")
            wk8 = projw.tile([P, 8, D], FP8, tag="wk", name="wk")
            for p4 in range(4):
                pr = slice(2 * p4, 2 * p4 + 2)
                nc.sync.dma_start(out=wk8[:, pr, :], in_=wk_v[:, pr, :])
                nc.sync.dma_start(out=x8_sb[:, pr, :], in_=x8_v[:, pr, :])
            xb_sb = pairload(projw, xb_ext, "xb", BF16)
            wv1_sb = pairload(projw, wv1_ext, "wv1", BF16)
            wq8 = pairload(projw, wq_ext, "wq", FP8)

            def kqt_proj(w8, cols, out_cb):
                # out[p(dout sub m), c] for c in cols; fp8 DoubleRow over d.
                # p4 outer so matmuls start as soon as sub-pair 0 arrives;
                # m in quarters of 2 to stay within 2 PSUM banks.
                for quarter in range(4):
                    ms = range(2 * quarter, 2 * quarter + 2)
                    accs = {
                        m: ppsum.tile([P, 256], F32, tag=f"proj{m % 2}", name=f"kq{m}")
                        for m in ms
                    }
                    for p4 in range(4):
                        for m in ms:
                            nc.tensor.matmul(
                                accs[m],
                                lhsT=w8[:, 2 * p4 : 2 * p4 + 2, m * P : (m + 1) * P],
                                rhs=x8_sb[:, 2 * p4 : 2 * p4 + 2, cols],
                                start=(p4 == 0),
                                stop=(p4 == 3),
                                perf_mode=DR,
                            )
                    for m in ms:
                        out_cb(m, accs[m])

            for g in range(NSLOTS):
                cols = slice(g * 256, (g + 1) * 256)
                kt_out = projout.tile([P, 8, 256], FP8, tag="kt_out", name="kto")
                kqt_proj(
                    wk8,
                    cols,
                    lambda m, acc: nc.vector.tensor_copy(out=kt_out[:, m, :], in_=acc),
                )
                nc.sync.dma_start(
                    out=kv_local[g, R_KT : R_KT + P].bitcast(FP8),
                    in_=kt_out.rearrange("p m c -> p (m c)"),
                )
                for blk in range(2):
                    v_out = projout.tile([P, D], BF16, tag="v_out", name="vo")
                    v8_out = projout.tile([P, D], FP8, tag="v8_out", name="v8o")
                    accs = [
                        vpsum.tile([P, 512], F32, tag="vproj", name=f"vp{h2}")
                        for h2 in range(2)
                    ]
                    bc = slice(g * 256 + blk * P, g * 256 + (blk + 1) * P)
                    for sub in range(8):
                        for h2 in range(2):
                            nc.tensor.matmul(
                                accs[h2],
                                lhsT=xb_sb[:, sub, bc],
                                rhs=wv1_sb[:, sub, h2 * 512 : (h2 + 1) * 512],
                                start=(sub == 0),
                                stop=(sub == 7),
                            )
                    for h2 in range(2):
                        nc.vector.tensor_copy(
                            out=v_out[:, h2 * 512 : (h2 + 1) * 512], in_=accs[h2]
                        )
                        nc.vector.tensor_copy(
                            out=v8_out[:, h2 * 512 : (h2 + 1) * 512], in_=accs[h2]
                        )
                    nc.sync.dma_start(
                        out=kv_local[g, R_V16 + blk * P : R_V16 + (blk + 1) * P],
                        in_=v_out,
                    )
                    nc.sync.dma_start(
                        out=kv_local[
                            g, R_V8 : R_V8 + P, blk * 512 : (blk + 1) * 512
                        ].bitcast(FP8),
                        in_=v8_out,
                    )
                    # colsum over this block's x rows -> for the delta-trick
                    # correction vectors at the consumers
                    with nc.allow_low_precision(reason="bf16 colsum feeds hi+lo split"):
                        nc.vector.tensor_reduce(
                            out=xsumb[:, :, 2 * g + blk : 2 * g + blk + 1],
                            in_=xb_sb[:, :, bc],
                            axis=AX.X,
                            op=ALU.add,
                        )
                cs = cpsum.tile([2, D], F32, tag="cs", name="cs")
                for sub in range(8):
                    for dh in range(2):
                        nc.tensor.matmul(
                            cs[:, dh * 512 : (dh + 1) * 512],
                            lhsT=xsumb[:, sub, 2 * g : 2 * g + 2],
                            rhs=wv1_sb[:, sub, dh * 512 : (dh + 1) * 512],
                            start=(sub == 0),
                            stop=(sub == 7),
                        )
                csb = projout.tile([2, D], BF16, tag="csb", name="csb")
                nc.vector.tensor_copy(out=csb, in_=cs)
                nc.sync.dma_start(out=kv_local[g, R_CS : R_CS + 2], in_=csb)
                nc.gpsimd.collective_compute(
                    "AllGather",
                    mybir.AluOpType.bypass,
                    replica_groups=[list(range(NCORES))],
                    ins=[kv_local[g]],
                    outs=[kv_gath[g]],
                )

            for s in range(NSLOTS):
                cols = slice(s * 256, (s + 1) * 256)
                kqt_proj(
                    wq8,
                    cols,
                    lambda m, acc: nc.vector.tensor_copy(
                        out=qt_sb[:, m, s * 256 : (s + 1) * 256], in_=acc
                    ),
                )

        # ---- attention ----------------------------------------------------
        with (
            tc.tile_pool(name="asingles", bufs=1) as asingles,
            tc.tile_pool(name="vpool", bufs=3) as vpool,
            tc.tile_pool(name="ptpool", bufs=4) as ptpool,
            tc.tile_pool(name="epool", bufs=2) as epool,
            tc.tile_pool(name="gpool", bufs=1) as gpool,
            tc.tile_pool(name="ltpsum", bufs=2, space="PSUM") as ltpsum,
            tc.tile_pool(name="hpsum", bufs=1, space="PSUM") as hpsum,
            tc.tile_pool(name="spsum", bufs=1, space="PSUM") as spsum,
        ):
            wv2_sb = wload(nc, asingles, wv2_ext, "wv2", BF16)
            masks_sb = asingles.tile([P, 64, 256], BF16)
            nc.sync.dma_start(
                out=masks_sb, in_=mask_ext[:].rearrange("j p c -> p j c")
            )
            # slot-level-0 kt and fp8 V stay SBUF-resident: they serve the
            # first 16 visits of every slot (phase A of slots 1-3, phase B
            # kt of slot 0).  Slot 0's bf16 V is streamed like other levels.
            kt0_sb = asingles.tile([P, 8, 8, 256], FP8)
            v80_sb = asingles.tile([P, 8, 2048], FP8)
            for t in range(8):
                nc.sync.dma_start(
                    out=kt0_sb[:, t].rearrange("p m c -> p (m c)"),
                    in_=kv_gath[0, t, R_KT : R_KT + P].bitcast(FP8),
                )
                nc.sync.dma_start(
                    out=v80_sb[:, t],
                    in_=kv_gath[0, t, R_V8 : R_V8 + P].bitcast(FP8),
                )

            # correction vectors: corr[s] = sum of colsum(V) over levels < s.
            # corrf is the fp32 master; corrh/corrl the bf16 hi+lo split used
            # by the rank-1 seed matmuls.  Levels accumulate incrementally at
            # each slot's phase-B start (gather s is complete by then).
            corrf = asingles.tile([1, NSLOTS, D], F32)
            corrh = asingles.tile([1, NSLOTS, D], BF16)
            corrl = asingles.tile([1, NSLOTS, D], BF16)
            nc.vector.memset(corrf[:, 0], 0.0)
            nc.vector.memset(corrh[:, 0], 0.0)
            nc.vector.memset(corrl[:, 0], 0.0)

            def corr_level(g):
                # fold gather level g's colsums into corr[g+1]
                csg = epool.tile([16, D], BF16, tag="csg", name=f"csg{g}")
                for b in range(2):
                    nc.sync.dma_start(
                        out=csg[8 * b : 8 * b + 8], in_=kv_gath[g, :, R_CS + b]
                    )
                for hf in range(2):
                    ch = slice(hf * 512, (hf + 1) * 512)
                    lvl = spsum.tile([1, 512], F32, tag="lvl", name="lvl")
                    nc.tensor.matmul(
                        lvl, lhsT=ones_sb[0:16, :], rhs=csg[:, ch], start=True, stop=True
                    )
                    nc.vector.tensor_add(
                        out=corrf[:, g + 1, ch], in0=corrf[:, g, ch], in1=lvl
                    )
                with nc.allow_low_precision(reason="bf16 hi+lo split of fp32 corr"):
                    nc.vector.tensor_copy(out=corrh[:, g + 1], in_=corrf[:, g + 1])
                    nc.vector.tensor_sub(
                        out=corrl[:, g + 1], in0=corrf[:, g + 1], in1=corrh[:, g + 1]
                    )

            def load_ktpair(t):
                g, r = t // 8, t % 8
                kt2 = vpool.tile([P, 2, 8, 256], FP8, tag="kt", name="kt2")
                nc.sync.dma_start(
                    out=kt2.rearrange("p a m c -> p a (m c)"),
                    in_=kv_gath[g, r : r + 2, R_KT : R_KT + P]
                    .bitcast(FP8)
                    .rearrange("a p c -> p a c"),
                )
                return kt2

            def load_tile_A(t, cache):
                # kt + fp8 V for interior visits
                if t < 8:
                    return kt0_sb[:, t], v80_sb[:, t].rearrange(
                        "p (b c) -> p b c", b=2
                    )
                if t % 2 == 0:
                    kt2 = load_ktpair(t)
                    v82 = vpool.tile([P, 2, 2048], FP8, tag="v8", name="v82")
                    nc.sync.dma_start(
                        out=v82,
                        in_=kv_gath[t // 8, t % 8 : t % 8 + 2, R_V8 : R_V8 + P]
                        .bitcast(FP8)
                        .rearrange("a p c -> p a c"),
                    )
                    cache["pair"] = (kt2, v82)
                kt2, v82 = cache["pair"]
                return kt2[:, t % 2], v82[:, t % 2].rearrange("p (b c) -> p b c", b=2)

            def load_tile_B(t, cache):
                # kt (cached for level 0) + streamed bf16 V for boundary visits
                if t % 2 == 0:
                    kt2 = None if t < 8 else load_ktpair(t)
                    v2 = vpool.tile([P, 2, 2, D], BF16, tag="v", name="v2")
                    g, r = t // 8, t % 8
                    for i in range(2):
                        nc.sync.dma_start(
                            out=v2[:, i],
                            in_=kv_gath[g, r + i, R_V16 : R_V16 + 2 * P].rearrange(
                                "(b p) d -> p b d", p=P
                            ),
                        )
                    cache["pair"] = (kt2, v2)
                kt2, v2 = cache["pair"]
                kt_t = kt0_sb[:, t] if t < 8 else kt2[:, t % 2]
                return kt_t, v2[:, t % 2]

            def logits(s, kt_t, b):
                lt = ltpsum.tile([P, 256], F32, tag="lt", name="lt")
                for p4 in range(4):
                    nc.tensor.matmul(
                        lt,
                        lhsT=kt_t[:, 2 * p4 : 2 * p4 + 2, b * P : (b + 1) * P],
                        rhs=qt_sb[:, 2 * p4 : 2 * p4 + 2, s * 256 : (s + 1) * 256],
                        start=(p4 == 0),
                        stop=(p4 == 3),
                        perf_mode=DR,
                    )
                return lt

            def pvA(pt8_t, v8_t, h, sums):
                for qc in range(2):
                    lhsT = pt8_t[:, :, qc * P : (qc + 1) * P]
                    for dh in range(2):
                        nc.tensor.matmul(
                            h[qc][:, dh, :],
                            lhsT=lhsT,
                            rhs=v8_t[:, :, dh * 512 : (dh + 1) * 512],
                            start=False,
                            stop=False,
                            perf_mode=DR,
                        )
                    nc.tensor.matmul(
                        sums[:, qc : qc + 1],
                        lhsT=lhsT,
                        rhs=ones8_2,
                        start=False,
                        stop=False,
                        perf_mode=DR,
                        skip_group_check=True,
                    )

            def pvB(s, j, lt, v_t, b, h, sums, jmax):
                pt = ptpool.tile([P, 256], BF16, tag="pt", name="pt")
                nc.scalar.activation(out=pt, in_=lt, func=AF.Exp, scale=EXP_SCALE)
                nc.vector.tensor_mul(out=pt, in0=pt, in1=masks_sb[:, j])
                for qc in range(2):
                    lhsT = pt[:, qc * P : (qc + 1) * P]
                    for dh in range(2):
                        nc.tensor.matmul(
                            h[qc][:, dh, :],
                            lhsT=lhsT,
                            rhs=v_t[:, b, dh * 512 : (dh + 1) * 512],
                            start=False,
                            stop=(j == jmax),
                        )
                    nc.tensor.matmul(
                        sums[:, qc : qc + 1],
                        lhsT=lhsT,
                        rhs=ones_sb,
                        start=False,
                        stop=(j == jmax),
                        skip_group_check=True,
                    )

            for s in range(NSLOTS):
                nv = 16 * (s + 1)
                jmax = nv - 1
                h = [
                    hpsum.tile([P, 2, 512], F32, tag=f"hq{qc}", name=f"h{qc}_{s}")
                    for qc in range(2)
                ]
                sums = spsum.tile([P, 2], F32, tag="sums", name="sums")
                # seeds: rowsum count for the interior positions, and the
                # exact ones@V part of the delta decomposition (hi+lo bf16)
                nc.tensor.matmul(
                    sums,
                    lhsT=ones_row,
                    rhs=cnt_sb[:, 2 * s : 2 * s + 2],
                    start=True,
                    stop=False,
                    skip_group_check=True,
                )
                for qc in range(2):
                    for dh in range(2):
                        nc.tensor.matmul(
                            h[qc][:, dh, :],
                            lhsT=ones_row,
                            rhs=corrh[:, s, dh * 512 : (dh + 1) * 512],
                            start=True,
                            stop=False,
                        )
                        nc.tensor.matmul(
                            h[qc][:, dh, :],
                            lhsT=ones_row,
                            rhs=corrl[:, s, dh * 512 : (dh + 1) * 512],
                            start=False,
                            stop=False,
                        )
                # phase A (interior, fully visible): fp8 delta PV per tile
                pend = None
                cache = {}
                for t in range(8 * s):
                    kt_t, v8_t = load_tile_A(t, cache)
                    lts = [logits(s, kt_t, b) for b in range(2)]
                    if pend is not None:
                        pvA(*pend, h, sums)
                    pt8_t = ptpool.tile([P, 2, 256], FP8, tag="pt8", name="pt8")
                    for b in range(2):
                        ptf = ptpool.tile([P, 256], F32, tag="ptf", name="ptf")
                        nc.scalar.activation(
                            out=ptf, in_=lts[b], func=AF.Exp, scale=EXP_SCALE
                        )
                        with nc.allow_low_precision(
                            reason="fp8 delta is the point of the decomposition"
                        ):
                            nc.vector.tensor_scalar_add(
                                out=pt8_t[:, b, :], in0=ptf, scalar1=-1.0
                            )
                    pend = (pt8_t, v8_t)
                if pend is not None:
                    pvA(*pend, h, sums)
                # phase B (boundary, masked): bf16 per-visit pipeline
                if s < 3:
                    corr_level(s)
                pendB = None
                cacheB = {}
                for t in range(8 * s, 8 * (s + 1)):
                    kt_t, v_t = load_tile_B(t, cacheB)
                    for b in range(2):
                        j = 2 * t + b
                        lt = logits(s, kt_t, b)
                        if pendB is not None:
                            pvB(s, *pendB, h, sums, jmax)
                        pendB = (j, lt, v_t, b)
                pvB(s, *pendB, h, sums, jmax)

                # ---- epilogue ----------------------------------------
                g_bf = []
                for qc in range(2):
                    recip = epool.tile([P, 1], F32, tag="recip", name="recip")
                    nc.vector.reciprocal(out=recip, in_=sums[:, qc : qc + 1])
                    g = gpool.tile([P, D], BF16, tag=f"g{qc}", name=f"g{qc}")
                    nc.scalar.activation(
                        out=g,
                        in_=h[qc].rearrange("p a b -> p (a b)"),
                        func=AF.Silu,
                        scale=recip,
                    )
                    g_bf.append(g)
                # transpose G -> gt [d-part, m, 256]; transposes borrow the
                # lt psum banks (free during the epilogue) via bitcast
                gt_sb = epool.tile([P, 8, 256], BF16, tag="gt", name="gt")
                for m in range(8):
                    for qc in range(2):
                        tp = ltpsum.tile([P, 256], F32, tag="lt", name="tp")
                        tpb = tp.bitcast(BF16)
                        nc.tensor.transpose(
                            tpb[:, :P],
                            g_bf[qc][:, m * P : (m + 1) * P],
                            ident_sb,
                        )
                        nc.vector.tensor_copy(
                            out=gt_sb[:, m, qc * P : (qc + 1) * P], in_=tpb[:, :P]
                        )
                # output projection: O[q, d] via lhsT = gt chunks
                for qc in range(2):
                    op = hpsum.tile([P, 2, 512], F32, tag=f"hq{qc}", name=f"o{qc}_{s}")
                    for m in range(8):
                        for dh in range(2):
                            nc.tensor.matmul(
                                op[:, dh, :],
                                lhsT=gt_sb[:, m, qc * P : (qc + 1) * P],
                                rhs=wv2_sb[:, m, dh * 512 : (dh + 1) * 512],
                                start=(m == 0),
                                stop=(m == 7),
                            )
                    oo = epool.tile([P, 2, 512], F32, tag="oo", name="oo")
                    nc.vector.tensor_copy(out=oo, in_=op)
                    nc.sync.dma_start(
                        out=o_ext[s, qc], in_=oo.rearrange("p a b -> p (a b)")
                    )

        singles_ctx.__exit__(None, None, None)

    nc.finalize()
    return nc


_NC_CACHE = {}


def get_nc():
    if "nc" not in _NC_CACHE:
        _NC_CACHE["nc"] = build_kernel()
    return _NC_CACHE["nc"]


def build_masks():
    """Masks for the last 16 visits of each slot, selected per core by
    k = 2c + 16s - j: k>=1 all-visible, k==0 upper-left triangle, k==-1
    shifted triangle, k<=-2 fully masked (padded visit)."""
    p = np.arange(P)[:, None]
    u = np.arange(256)[None, :]
    m_ones = np.ones((P, 256), np.float32)
    m0 = (p <= u).astype(np.float32)
    m1 = (p <= u - P).astype(np.float32)
    m_zero = np.zeros((P, 256), np.float32)
    canon = np.stack([m_zero, m1, m0, m_ones]).astype(ml_dtypes.bfloat16)

    out = []
    for c in range(NCORES):
        sel = []
        for s in range(NSLOTS):
            for j in range(16 * s, 16 * (s + 1)):
                k = 2 * c + 16 * s - j
                sel.append(min(max(k, -2), 1) + 2)
        out.append(canon[np.array(sel, np.int64)])
    return out  # list of [64, 128, 256] bf16


def build_in_maps(x, wq, wk, wv1, wv2):
    bf = ml_dtypes.bfloat16
    f8 = ml_dtypes.float8_e4m3
    xT = np.ascontiguousarray(np.asarray(x, np.float32).T)
    masks = build_masks()
    w = {
        "wq": (np.asarray(wq, np.float32) * WSCALE).astype(f8),
        "wk": (np.asarray(wk, np.float32) * WSCALE).astype(f8),
        "wv1": np.asarray(wv1, np.float32).astype(bf),
        "wv2": np.asarray(wv2, np.float32).astype(bf),
    }
    in_maps = []
    for c in range(NCORES):
        xq_c = np.ascontiguousarray(
            np.concatenate(
                [
                    xT[:, 256 * (c + 8 * s) : 256 * (c + 8 * s) + 256]
                    for s in range(NSLOTS)
                ],
                axis=1,
            )
        )
        in_maps.append(
            {
                "xb": xq_c.astype(bf),
                "x8": xq_c.astype(f8),
                "masks": masks[c],
                **w,
            }
        )
    return in_maps


def assemble_out(results):
    out = np.empty((SEQ, D), np.float32)
    for c in range(NCORES):
        o = results[c]["o"]  # [4, 2, 128, 1024]
        for s in range(NSLOTS):
            r0 = 256 * (c + 8 * s)
            out[r0 : r0 + P, :] = o[s, 0]
            out[r0 + P : r0 + 256, :] = o[s, 1]
    return out


def kernel(x, wq, wk, wv1, wv2):
    in_maps = build_in_maps(x, wq, wk, wv1, wv2)
    nc = get_nc()
    res = run_bass_kernel_spmd(nc, in_maps, list(range(NCORES)))
    return assemble_out(res.results)


# revision 39
# speedup vs baseline: 1.0324x; 1.0324x over previous
"""Trainium2 Bass kernel for nn_MemoryAttention (causal single-head attention
with SiLU-gated output projection), sequence-parallel across 8 NeuronCores.

Strategy (per core c):
  - q rows owned: 4 slots of 256 rows: tile t = c + 8*s (strided assignment
    balances causal work; every core runs an identical instruction stream).
  - fp8e4 (DoubleRow, 2x PE rate) for the Q/K projections and the QK^T
    logits: softmax logits are tiny (~+-0.1 after 1/32 scaling), so ~5%
    quantization on q/k perturbs attention weights by well under 1%.
    wq/wk are pre-scaled by 64 host-side (w std 0.01 would land in the
    fp8 subnormal range); the exp() scale folds the 64*64 back out.
  - Interior ("phase A", fully-visible) visits also run PV in fp8
    DoubleRow via the delta decomposition P = 1 + delta: h = prefix
    column-sums of V (exact, injected as rank-1 seed matmuls from
    colsum(X) @ wv1 colsums that ride the gather) + delta @ V8.  fp8
    errors then scale by |delta| ~ 0.15 instead of 1.  Boundary
    ("phase B", last 16 visits of each slot, masked) visits stay bf16.
  - Each core projects KT(fp8)/V(bf16+fp8) for its own tile of slot-level
    g, then immediately AllGathers that slot-level (4 small pipelined
    collectives) so slot g's kv arrives while earlier slots compute.
    Slot-level 0 stays SBUF-resident (serves the first 16 visits of
    every slot).
  - Slot epilogue: H / rowsums, SiLU, PE-transpose of G, output proj.
"""

import numpy as np
import ml_dtypes

import concourse.bass as bass
import concourse.tile as tile
from concourse import bacc, mybir
from concourse.bass_utils import run_bass_kernel_spmd
from concourse.masks import make_identity

P = 128
D = 1024
SEQ = 8192
NCORES = 8
NSLOTS = 4
WSCALE = 64.0
EXP_SCALE = 0.03125 / (WSCALE * WSCALE)

# kv payload rows (units of [., 1024] bf16): kt fp8 | v bf16 | v fp8 | colsum
R_KT = 0
R_V16 = P
R_V8 = 3 * P
R_CS = 4 * P
R_TOT = 4 * P + 1

F32 = mybir.dt.float32
BF16 = mybir.dt.bfloat16
FP8 = mybir.dt.float8e4
AF = mybir.ActivationFunctionType
DR = mybir.MatmulPerfMode.DoubleRow
AX = mybir.AxisListType
ALU = mybir.AluOpType


def build_kernel():
    nc = bacc.Bacc(None, target_bir_lowering=False, num_devices=NCORES)

    xb_ext = nc.declare_dram_parameter("xb", [D, D], BF16, isOutput=False)
    x8_ext = nc.declare_dram_parameter("x8", [D, D], FP8, isOutput=False)
    wq_ext = nc.declare_dram_parameter("wq", [D, D], FP8, isOutput=False)
    wk_ext = nc.declare_dram_parameter("wk", [D, D], FP8, isOutput=False)
    wv1_ext = nc.declare_dram_parameter("wv1", [D, D], BF16, isOutput=False)
    wv2_ext = nc.declare_dram_parameter("wv2", [D, D], BF16, isOutput=False)
    mask_ext = nc.declare_dram_parameter("masks", [64, P, 256], BF16, isOutput=False)
    o_ext = nc.declare_dram_parameter("o", [NSLOTS, 2, P, D], F32, isOutput=True)

    kv_local = nc.dram_tensor("kv_local", [NSLOTS, R_TOT, D], BF16)
    kvB_gath = nc.dram_tensor(
        "kvB_gath", [NSLOTS, NCORES, R_V8, D], BF16, addr_space="Shared"
    )
    kvA_gath = nc.dram_tensor(
        "kvA_gath", [3, NCORES, R_TOT - R_V8, D], BF16, addr_space="Shared"
    )

    def wload(nc, pool, ext, tag, dt):
        t = pool.tile([P, 8, D], dt, tag=tag, name=tag)
        nc.sync.dma_start(out=t, in_=ext[:].rearrange("(sub p) s -> p sub s", p=P))
        return t

    with tile.TileContext(nc) as tc:
        singles_ctx = tc.tile_pool(name="singles", bufs=1)
        singles = singles_ctx.__enter__()

        ones_sb = singles.tile([P, 1], BF16)
        nc.vector.memset(ones_sb, 1.0)
        ones_row = singles.tile([1, P], BF16)
        nc.vector.memset(ones_row, 1.0)
        ones8_2 = singles.tile([P, 2, 1], FP8)
        nc.vector.memset(ones8_2, 1.0)
        cnt_sb = singles.tile([1, 2 * NSLOTS], BF16)
        for s in range(NSLOTS):
            nc.vector.memset(cnt_sb[:, 2 * s : 2 * s + 2], float(2048 * s))
        ident_sb = singles.tile([P, P], BF16)
        make_identity(nc, ident_sb)
        qt_sb = singles.tile([P, 8, D], FP8)

        with (
            tc.tile_pool(name="projw", bufs=1) as projw,
            tc.tile_pool(name="projout", bufs=4) as projout,
            tc.tile_pool(name="ppsum", bufs=1, space="PSUM") as ppsum,
            tc.tile_pool(name="vpsum", bufs=4, space="PSUM") as vpsum,
            tc.tile_pool(name="cpsum", bufs=1, space="PSUM") as cpsum,
        ):
            # sub-pair-chunked loads so the first DoubleRow matmuls (which
            # consume one sub-pair at a time) start after two small DMAs
            def pairload(pool, ext, tag, dt):
                t = pool.tile([P, 8, D], dt, tag=tag, name=tag)
                v = ext[:].rearrange("(sub p) s -> p sub s", p=P)
                for p4 in range(4):
                    nc.sync.dma_start(
                        out=t[:, 2 * p4 : 2 * p4 + 2, :], in_=v[:, 2 * p4 : 2 * p4 + 2, :]
                    )
                return t

            x8_v = x8_ext[:].rearrange("(sub p) s -> p sub s", p=P)
            wk_v = wk_ext[:].rearrange("(sub p) s -> p sub s", p=P)
            x8_sb = projw.tile([P, 8, D], FP8, tag="x8", name="x8")
            wk8 = projw.tile([P, 8, D], FP8, tag="wk", name="wk")
            for p4 in range(4):
                pr = slice(2 * p4, 2 * p4 + 2)
                nc.sync.dma_start(out=wk8[:, pr, :], in_=wk_v[:, pr, :])
                nc.sync.dma_start(out=x8_sb[:, pr, :], in_=x8_v[:, pr, :])
            xb_sb = pairload(projw, xb_ext, "xb", BF16)
            wv1_sb = pairload(projw, wv1_ext, "wv1", BF16)
            wq8 = pairload(projw, wq_ext, "wq", FP8)

            def kqt_proj(w8, cols, out_cb):
                # out[p(dout sub m), c] for c in cols; fp8 DoubleRow over d.
                # p4 outer so matmuls start as soon as sub-pair 0 arrives;
                # m in quarters of 2 to stay within 2 PSUM banks.
                for quarter in range(4):
                    ms = range(2 * quarter, 2 * quarter + 2)
                    accs = {
                        m: ppsum.tile([P, 256], F32, tag=f"proj{m % 2}", name=f"kq{m}")
                        for m in ms
                    }
                    for p4 in range(4):
                        for m in ms:
                            nc.tensor.matmul(
                                accs[m],
                                lhsT=w8[:, 2 * p4 : 2 * p4 + 2, m * P : (m + 1) * P],
                                rhs=x8_sb[:, 2 * p4 : 2 * p4 + 2, cols],
                                start=(p4 == 0),
                                stop=(p4 == 3),
                                perf_mode=DR,
                            )
                    for m in ms:
                        out_cb(m, accs[m])

            def gather(in_ap, out_ap):
                nc.gpsimd.collective_compute(
                    "AllGather",
                    mybir.AluOpType.bypass,
                    replica_groups=[list(range(NCORES))],
                    ins=[in_ap],
                    outs=[out_ap],
                )

            for g in range(NSLOTS):
                cols = slice(g * 256, (g + 1) * 256)
                kt_out = projout.tile([P, 8, 256], FP8, tag="kt_out", name="kto")
                kqt_proj(
                    wk8,
                    cols,
                    lambda m, acc: nc.vector.tensor_copy(out=kt_out[:, m, :], in_=acc),
                )
                nc.sync.dma_start(
                    out=kv_local[g, R_KT : R_KT + P].bitcast(FP8),
                    in_=kt_out.rearrange("p m c -> p (m c)"),
                )
                cs = cpsum.tile([1, D], F32, tag="cs", name="cs")
                v8_outs = []
                for blk in range(2):
                    v_out = projout.tile([P, D], BF16, tag="v_out", name="vo")
                    v8_out = projout.tile([P, D], FP8, tag="v8_out", name="v8o")
                    accs = [
                        vpsum.tile([P, 512], F32, tag="vproj", name=f"vp{h2}")
                        for h2 in range(2)
                    ]
                    bc = slice(g * 256 + blk * P, g * 256 + (blk + 1) * P)
                    for sub in range(8):
                        for h2 in range(2):
                            nc.tensor.matmul(
                                accs[h2],
                                lhsT=xb_sb[:, sub, bc],
                                rhs=wv1_sb[:, sub, h2 * 512 : (h2 + 1) * 512],
                                start=(sub == 0),
                                stop=(sub == 7),
                            )
                    for h2 in range(2):
                        nc.vector.tensor_copy(
                            out=v_out[:, h2 * 512 : (h2 + 1) * 512], in_=accs[h2]
                        )
                        if g < 3:
                            nc.vector.tensor_copy(
                                out=v8_out[:, h2 * 512 : (h2 + 1) * 512], in_=accs[h2]
                            )
                            # colsum(V) for the delta-trick corrections; own
                            # blocks pre-summed so the wire carries one row
                            nc.tensor.matmul(
                                cs[:, h2 * 512 : (h2 + 1) * 512],
                                lhsT=ones_sb,
                                rhs=v_out[:, h2 * 512 : (h2 + 1) * 512],
                                start=(blk == 0),
                                stop=(blk == 1),
                            )
                    nc.sync.dma_start(
                        out=kv_local[g, R_V16 + blk * P : R_V16 + (blk + 1) * P],
                        in_=v_out,
                    )
                    v8_outs.append(v8_out)
                # phase-B payload (kt + v16) gathers first: it is consumed
                # earlier (slot g's boundary visits) than the phase-A payload
                # (v8 + colsum, consumed by slots > g).  Level 3 has no
                # phase-A consumers at all.
                gather(kv_local[g, 0:R_V8], kvB_gath[g])
                if g < 3:
                    for blk in range(2):
                        nc.sync.dma_start(
                            out=kv_local[
                                g, R_V8 : R_V8 + P, blk * 512 : (blk + 1) * 512
                            ].bitcast(FP8),
                            in_=v8_outs[blk],
                        )
                    csb = projout.tile([1, D], BF16, tag="csb", name="csb")
                    nc.vector.tensor_copy(out=csb, in_=cs)
                    nc.sync.dma_start(out=kv_local[g, R_CS : R_CS + 1], in_=csb)
                    gather(kv_local[g, R_V8:R_TOT], kvA_gath[g])

            for s in range(NSLOTS):
                cols = slice(s * 256, (s + 1) * 256)
                kqt_proj(
                    wq8,
                    cols,
                    lambda m, acc: nc.vector.tensor_copy(
                        out=qt_sb[:, m, s * 256 : (s + 1) * 256], in_=acc
                    ),
                )

        # ---- attention ----------------------------------------------------
        with (
            tc.tile_pool(name="asingles", bufs=1) as asingles,
            tc.tile_pool(name="vpool", bufs=3) as vpool,
            tc.tile_pool(name="ptpool", bufs=4) as ptpool,
            tc.tile_pool(name="epool", bufs=2) as epool,
            tc.tile_pool(name="gpool", bufs=1) as gpool,
            tc.tile_pool(name="ltpsum", bufs=2, space="PSUM") as ltpsum,
            tc.tile_pool(name="hpsum", bufs=1, space="PSUM") as hpsum,
            tc.tile_pool(name="spsum", bufs=1, space="PSUM") as spsum,
        ):
            wv2_sb = wload(nc, asingles, wv2_ext, "wv2", BF16)
            masks_sb = asingles.tile([P, 64, 256], BF16)
            nc.sync.dma_start(
                out=masks_sb, in_=mask_ext[:].rearrange("j p c -> p j c")
            )
            # slot-level-0 kt and fp8 V stay SBUF-resident: they serve the
            # first 16 visits of every slot (phase A of slots 1-3, phase B
            # kt of slot 0).  Slot 0's bf16 V is streamed like other levels.
            kt0_sb = asingles.tile([P, 8, 8, 256], FP8)
            v80_sb = asingles.tile([P, 8, 2048], FP8)
            for t in range(8):
                nc.sync.dma_start(
                    out=kt0_sb[:, t].rearrange("p m c -> p (m c)"),
                    in_=kvB_gath[0, t, 0:P].bitcast(FP8),
                )
                nc.sync.dma_start(
                    out=v80_sb[:, t],
                    in_=kvA_gath[0, t, 0:P].bitcast(FP8),
                )

            # correction vectors: corr[s] = sum of colsum(V) over levels < s.
            # corrf is the fp32 master; corrh/corrl the bf16 hi+lo split used
            # by the rank-1 seed matmuls.  Levels accumulate incrementally at
            # each slot's phase-B start (gather s is complete by then).
            corrf = asingles.tile([1, NSLOTS, D], F32)
            corrh = asingles.tile([1, NSLOTS, D], BF16)
            corrl = asingles.tile([1, NSLOTS, D], BF16)
            nc.vector.memset(corrf[:, 0], 0.0)
            nc.vector.memset(corrh[:, 0], 0.0)
            nc.vector.memset(corrl[:, 0], 0.0)

            def corr_level(g):
                # fold gather level g's colsums into corr[g+1]
                csg = epool.tile([8, D], BF16, tag="csg", name=f"csg{g}")
                nc.sync.dma_start(out=csg, in_=kvA_gath[g, :, P])
                for hf in range(2):
                    ch = slice(hf * 512, (hf + 1) * 512)
                    lvl = spsum.tile([1, 512], F32, tag="lvl", name="lvl")
                    nc.tensor.matmul(
                        lvl, lhsT=ones_sb[0:8, :], rhs=csg[:, ch], start=True, stop=True
                    )
                    nc.vector.tensor_add(
                        out=corrf[:, g + 1, ch], in0=corrf[:, g, ch], in1=lvl
                    )
                with nc.allow_low_precision(reason="bf16 hi+lo split of fp32 corr"):
                    nc.vector.tensor_copy(out=corrh[:, g + 1], in_=corrf[:, g + 1])
                    nc.vector.tensor_sub(
                        out=corrl[:, g + 1], in0=corrf[:, g + 1], in1=corrh[:, g + 1]
                    )

            def load_ktpair(t):
                g, r = t // 8, t % 8
                kt2 = vpool.tile([P, 2, 8, 256], FP8, tag="kt", name="kt2")
                nc.sync.dma_start(
                    out=kt2.rearrange("p a m c -> p a (m c)"),
                    in_=kvB_gath[g, r : r + 2, 0:P]
                    .bitcast(FP8)
                    .rearrange("a p c -> p a c"),
                )
                return kt2

            def load_tile_A(t, cache):
                # kt + fp8 V for interior visits
                if t < 8:
                    return kt0_sb[:, t], v80_sb[:, t].rearrange(
                        "p (b c) -> p b c", b=2
                    )
                if t % 2 == 0:
                    kt2 = load_ktpair(t)
                    v82 = vpool.tile([P, 2, 2048], FP8, tag="v8", name="v82")
                    nc.sync.dma_start(
                        out=v82,
                        in_=kvA_gath[t // 8, t % 8 : t % 8 + 2, 0:P]
                        .bitcast(FP8)
                        .rearrange("a p c -> p a c"),
                    )
                    cache["pair"] = (kt2, v82)
                kt2, v82 = cache["pair"]
                return kt2[:, t % 2], v82[:, t % 2].rearrange("p (b c) -> p b c", b=2)

            def load_tile_B(t, cache):
                # kt (cached for level 0) + streamed bf16 V for boundary visits
                if t % 2 == 0:
                    kt2 = None if t < 8 else load_ktpair(t)
                    v2 = vpool.tile([P, 2, 2, D], BF16, tag="v", name="v2")
                    g, r = t // 8, t % 8
                    for i in range(2):
                        nc.sync.dma_start(
                            out=v2[:, i],
                            in_=kvB_gath[g, r + i, P : 3 * P].rearrange(
                                "(b p) d -> p b d", p=P
                            ),
                        )
                    cache["pair"] = (kt2, v2)
                kt2, v2 = cache["pair"]
                kt_t = kt0_sb[:, t] if t < 8 else kt2[:, t % 2]
                return kt_t, v2[:, t % 2]

            def logits(s, kt_t, b):
                lt = ltpsum.tile([P, 256], F32, tag="lt", name="lt")
                for p4 in range(4):
                    nc.tensor.matmul(
                        lt,
                        lhsT=kt_t[:, 2 * p4 : 2 * p4 + 2, b * P : (b + 1) * P],
                        rhs=qt_sb[:, 2 * p4 : 2 * p4 + 2, s * 256 : (s + 1) * 256],
                        start=(p4 == 0),
                        stop=(p4 == 3),
                        perf_mode=DR,
                    )
                return lt

            def pvA(pt8_t, v8_t, h, sums):
                for qc in range(2):
                    lhsT = pt8_t[:, :, qc * P : (qc + 1) * P]
                    for dh in range(2):
                        nc.tensor.matmul(
                            h[qc][:, dh, :],
                            lhsT=lhsT,
                            rhs=v8_t[:, :, dh * 512 : (dh + 1) * 512],
                            start=False,
                            stop=False,
                            perf_mode=DR,
                        )
                    nc.tensor.matmul(
                        sums[:, qc : qc + 1],
                        lhsT=lhsT,
                        rhs=ones8_2,
                        start=False,
                        stop=False,
                        perf_mode=DR,
                        skip_group_check=True,
                    )

            def pvB(s, j, lt, v_t, b, h, sums, jmax):
                pt = ptpool.tile([P, 256], BF16, tag="pt", name="pt")
                nc.scalar.activation(out=pt, in_=lt, func=AF.Exp, scale=EXP_SCALE)
                nc.vector.tensor_mul(out=pt, in0=pt, in1=masks_sb[:, j])
                for qc in range(2):
                    lhsT = pt[:, qc * P : (qc + 1) * P]
                    for dh in range(2):
                        nc.tensor.matmul(
                            h[qc][:, dh, :],
                            lhsT=lhsT,
                            rhs=v_t[:, b, dh * 512 : (dh + 1) * 512],
                            start=False,
                            stop=(j == jmax),
                        )
                    nc.tensor.matmul(
                        sums[:, qc : qc + 1],
                        lhsT=lhsT,
                        rhs=ones_sb,
                        start=False,
                        stop=(j == jmax),
                        skip_group_check=True,
                    )

            for s in range(NSLOTS):
                nv = 16 * (s + 1)
                jmax = nv - 1
                h = [
                    hpsum.tile([P, 2, 512], F32, tag=f"hq{qc}", name=f"h{qc}_{s}")
                    for qc in range(2)
                ]
                sums = spsum.tile([P, 2], F32, tag="sums", name="sums")
                # seeds: rowsum count for the interior positions, and the
                # exact ones@V part of the delta decomposition (hi+lo bf16)
                nc.tensor.matmul(
                    sums,
                    lhsT=ones_row,
                    rhs=cnt_sb[:, 2 * s : 2 * s + 2],
                    start=True,
                    stop=False,
                    skip_group_check=True,
                )
                for qc in range(2):
                    for dh in range(2):
                        nc.tensor.matmul(
                            h[qc][:, dh, :],
                            lhsT=ones_row,
                            rhs=corrh[:, s, dh * 512 : (dh + 1) * 512],
                            start=True,
                            stop=False,
                        )
                        nc.tensor.matmul(
                            h[qc][:, dh, :],
                            lhsT=ones_row,
                            rhs=corrl[:, s, dh * 512 : (dh + 1) * 512],
                            start=False,
                            stop=False,
                        )
                # phase A (interior, fully visible): fp8 delta PV per tile,
                # 2-tile lookahead so the exp -> fp8-delta chain has slack
                pend = []
                cache = {}
                for t in range(8 * s):
                    kt_t, v8_t = load_tile_A(t, cache)
                    lts = [logits(s, kt_t, b) for b in range(2)]
                    if len(pend) == 2:
                        pvA(*pend.pop(0), h, sums)
                    pt8_t = ptpool.tile([P, 2, 256], FP8, tag="pt8", name="pt8")
                    for b in range(2):
                        ptf = ptpool.tile([P, 256], F32, tag="ptf", name="ptf")
                        nc.scalar.activation(
                            out=ptf, in_=lts[b], func=AF.Exp, scale=EXP_SCALE
                        )
                        with nc.allow_low_precision(
                            reason="fp8 delta is the point of the decomposition"
                        ):
                            nc.vector.tensor_scalar_add(
                                out=pt8_t[:, b, :], in0=ptf, scalar1=-1.0
                            )
                    pend.append((pt8_t, v8_t))
                for pe in pend:
                    pvA(*pe, h, sums)
                # phase B (boundary, masked): bf16 per-visit pipeline
                pendB = None
                cacheB = {}
                for t in range(8 * s, 8 * (s + 1)):
                    kt_t, v_t = load_tile_B(t, cacheB)
                    for b in range(2):
                        j = 2 * t + b
                        lt = logits(s, kt_t, b)
                        if pendB is not None:
                            pvB(s, *pendB, h, sums, jmax)
                        pendB = (j, lt, v_t, b)
                pvB(s, *pendB, h, sums, jmax)
                # fold this slot's colsum level into corr (data arrived with
                # this slot's phase-B gather; result needed at slot s+1 seeds)
                if s < 3:
                    corr_level(s)

                # ---- epilogue ----------------------------------------
                g_bf = []
                for qc in range(2):
                    recip = epool.tile([P, 1], F32, tag="recip", name="recip")
                    nc.vector.reciprocal(out=recip, in_=sums[:, qc : qc + 1])
                    g = gpool.tile([P, D], BF16, tag=f"g{qc}", name=f"g{qc}")
                    nc.scalar.activation(
                        out=g,
                        in_=h[qc].rearrange("p a b -> p (a b)"),
                        func=AF.Silu,
                        scale=recip,
                    )
                    g_bf.append(g)
                # transpose G -> gt [d-part, m, 256]; transposes borrow the
                # lt psum banks (free during the epilogue) via bitcast
                gt_sb = epool.tile([P, 8, 256], BF16, tag="gt", name="gt")
                for m in range(8):
                    for qc in range(2):
                        tp = ltpsum.tile([P, 256], F32, tag="lt", name="tp")
                        tpb = tp.bitcast(BF16)
                        nc.tensor.transpose(
                            tpb[:, :P],
                            g_bf[qc][:, m * P : (m + 1) * P],
                            ident_sb,
                        )
                        nc.vector.tensor_copy(
                            out=gt_sb[:, m, qc * P : (qc + 1) * P], in_=tpb[:, :P]
                        )
                # output projection: O[q, d] via lhsT = gt chunks
                for qc in range(2):
                    op = hpsum.tile([P, 2, 512], F32, tag=f"hq{qc}", name=f"o{qc}_{s}")
                    for m in range(8):
                        for dh in range(2):
                            nc.tensor.matmul(
                                op[:, dh, :],
                                lhsT=gt_sb[:, m, qc * P : (qc + 1) * P],
                                rhs=wv2_sb[:, m, dh * 512 : (dh + 1) * 512],
                                start=(m == 0),
                                stop=(m == 7),
                            )
                    oo = epool.tile([P, 2, 512], F32, tag="oo", name="oo")
                    nc.vector.tensor_copy(out=oo, in_=op)
                    nc.sync.dma_start(
                        out=o_ext[s, qc], in_=oo.rearrange("p a b -> p (a b)")
                    )

        singles_ctx.__exit__(None, None, None)

    nc.finalize()
    return nc


_NC_CACHE = {}


def get_nc():
    if "nc" not in _NC_CACHE:
        _NC_CACHE["nc"] = build_kernel()
    return _NC_CACHE["nc"]


def build_masks():
    """Masks for the last 16 visits of each slot, selected per core by
    k = 2c + 16s - j: k>=1 all-visible, k==0 upper-left triangle, k==-1
    shifted triangle, k<=-2 fully masked (padded visit)."""
    p = np.arange(P)[:, None]
    u = np.arange(256)[None, :]
    m_ones = np.ones((P, 256), np.float32)
    m0 = (p <= u).astype(np.float32)
    m1 = (p <= u - P).astype(np.float32)
    m_zero = np.zeros((P, 256), np.float32)
    canon = np.stack([m_zero, m1, m0, m_ones]).astype(ml_dtypes.bfloat16)

    out = []
    for c in range(NCORES):
        sel = []
        for s in range(NSLOTS):
            for j in range(16 * s, 16 * (s + 1)):
                k = 2 * c + 16 * s - j
                sel.append(min(max(k, -2), 1) + 2)
        out.append(canon[np.array(sel, np.int64)])
    return out  # list of [64, 128, 256] bf16


def build_in_maps(x, wq, wk, wv1, wv2):
    bf = ml_dtypes.bfloat16
    f8 = ml_dtypes.float8_e4m3
    xT = np.ascontiguousarray(np.asarray(x, np.float32).T)
    masks = build_masks()
    w = {
        "wq": (np.asarray(wq, np.float32) * WSCALE).astype(f8),
        "wk": (np.asarray(wk, np.float32) * WSCALE).astype(f8),
        "wv1": np.asarray(wv1, np.float32).astype(bf),
        "wv2": np.asarray(wv2, np.float32).astype(bf),
    }
    in_maps = []
    for c in range(NCORES):
        xq_c = np.ascontiguousarray(
            np.concatenate(
                [
                    xT[:, 256 * (c + 8 * s) : 256 * (c + 8 * s) + 256]
                    for s in range(NSLOTS)
                ],
                axis=1,
            )
        )
        in_maps.append(
            {
                "xb": xq_c.astype(bf),
                "x8": xq_c.astype(f8),
                "masks": masks[c],
                **w,
            }
        )
    return in_maps


def assemble_out(results):
    out = np.empty((SEQ, D), np.float32)
    for c in range(NCORES):
        o = results[c]["o"]  # [4, 2, 128, 1024]
        for s in range(NSLOTS):
            r0 = 256 * (c + 8 * s)
            out[r0 : r0 + P, :] = o[s, 0]
            out[r0 + P : r0 + 256, :] = o[s, 1]
    return out


def kernel(x, wq, wk, wv1, wv2):
    in_maps = build_in_maps(x, wq, wk, wv1, wv2)
    nc = get_nc()
    res = run_bass_kernel_spmd(nc, in_maps, list(range(NCORES)))
    return assemble_out(res.results)
